# revision 42
# baseline (speedup 1.0000x reference)
"""AdaptiveDCA Trainium2 kernel: 4-branch dilated 3x3 attention with gated concat.

Sharding: data-parallel over batch B=8 across 8 NeuronCores (1 image/core).
Per-core layout: channels on partitions, flattened HW on free axis.
  - qkv projection: TensorE GEMM, w^T stationary, x streaming (bf16).
  - K/V stored zero-padded (128, 80*80) so every 3x3 dilated tap is a strided
    in-bounds slice; OOB taps give logit 0 / value 0 == reference semantics.
  - logits: single fused DVE q*k_shift mul (5-dim AP covering all 9 taps)
    + TensorE block-diag ones matmul partition-reduce (logits broadcast 64x).
  - softmax: ScalarE exp on tap-pairs (PSUM pair tiles), denominator via a
    compact side-channel: DMA-gather partitions {0,64} of E into [18, NQ],
    then ONE TensorE matmul with a per-branch gate-scaled [18, 128] head-map
    stationary gives D/gate_b broadcast to all 128 channels; DVE reciprocal
    yields Rb = gate_b/D so the final normalize needs no gate dependency.
  - AV: fused DVE E*v_shift mul + TensorE identity-matmul PSUM accumulation;
    out = AV * Rb (plain DVE mul).
  - gate pooling on ScalarE accum + DVE reduces spread over idle slots
    (no PE matmuls blocking on the full x load).
  - Scheduling: one global GEMM-chunk queue in dependency-need order pulled
    at a tuned per-step rate, so b0 attention starts ~4us in; branches 2+3
    attention steps are interleaved (offset 2) to kill the no-filler tail.
"""

import os
import sys

sys.path.insert(0, "/opt/trn_rl_repo")

import numpy as np
import ml_dtypes

import concourse.bass as bass
import concourse.tile as tile
from concourse import bacc, mybir
from concourse.bass_utils import run_bass_kernel_spmd

F32 = mybir.dt.float32
BF16 = mybir.dt.bfloat16
AF = mybir.ActivationFunctionType
ALU = mybir.AluOpType

P = 128
N = 4096          # 64*64
H = W = 64
PAD = 8
WP = 80           # padded width/height
NPAD = WP * WP    # 6400
DILS = (1, 2, 4, 8)
NQ = 512          # positions per attention step (8 image rows)
QROWS = NQ // W   # image rows per attention slice
NSTEP = N // NQ   # 8 steps per branch
EPITCH = 9 * NQ   # Edall per-partition elements

# which tap-row (di) groups of the QK / AV products run on GpSimd
QK_POOL_DI = (1,)
AV_POOL_DI = ()

_CACHE = {}


def _sub(ap, off_elems, dims):
    """Manual free-dim sub-AP of a tile AP (keeps partition dim)."""
    part = list(ap.ap[0])
    return bass.AP(
        tensor=ap.tensor,
        offset=ap.offset + off_elems,
        ap=[part] + [[s, n] for s, n in dims],
    )


def _need_order(b):
    """GEMM chunks of branch b in the order attention consumes them.
    Step t's prod needs k chunks <= t+1 and q chunk t (emitted one step
    early); back(t) needs v chunks <= t+1."""
    seq = [("k", 0), ("k", 1), ("q", 0), ("v", 0)]
    for t in range(6):
        seq += [("k", t + 2), ("v", t + 1), ("q", t + 1)]
    seq += [("v", 7), ("q", 7)]
    assert len(seq) == 24
    return [(b, kind, ch) for kind, ch in seq]


def _build():
    nc = bacc.Bacc("TRN2", target_bir_lowering=False, debug=False, num_devices=8)

    x_d = nc.dram_tensor("x", [512, N], BF16, kind="ExternalInput").ap()
    wq_d = nc.dram_tensor("wqkvT", [512, 1536], BF16, kind="ExternalInput").ap()
    wg_d = nc.dram_tensor("wgT", [512, 4], BF16, kind="ExternalInput").ap()
    bg_d = nc.dram_tensor("bg", [4, 1], F32, kind="ExternalInput").ap()
    bd_d = nc.dram_tensor("blkdiag", [128, 128], BF16, kind="ExternalInput").ap()
    sh_d = nc.dram_tensor("selhead", [18, 128], BF16, kind="ExternalInput").ap()
    id_d = nc.dram_tensor("ident", [128, 128], BF16, kind="ExternalInput").ap()
    out_d = nc.dram_tensor("out", [512, N], F32, kind="ExternalOutput").ap()

    from contextlib import ExitStack

    with tile.TileContext(nc) as tc, ExitStack() as ctx:
        consts = ctx.enter_context(tc.tile_pool(name="consts", bufs=1))
        xpool = ctx.enter_context(tc.tile_pool(name="xp", bufs=1))
        qpool = ctx.enter_context(tc.tile_pool(name="qp", bufs=2))
        kpool = ctx.enter_context(tc.tile_pool(name="kp", bufs=2))
        vpool = ctx.enter_context(tc.tile_pool(name="vp", bufs=2))
        # PSUM: pair-tiles (2 banks x2) for logits, 1-bank x2 for AV/D,
        # 1-bank x2 for gemm chunks = 8 banks total
        ldps = ctx.enter_context(tc.tile_pool(name="ldps", bufs=2, space="PSUM"))
        sps = ctx.enter_context(tc.tile_pool(name="sps", bufs=2, space="PSUM"))
        gps = ctx.enter_context(tc.tile_pool(name="gps", bufs=2, space="PSUM"))
        epool = ctx.enter_context(tc.tile_pool(name="ep", bufs=3))
        ecpool = ctx.enter_context(tc.tile_pool(name="ecp", bufs=2))
        prodp = ctx.enter_context(tc.tile_pool(name="prodp", bufs=3))
        pvp = ctx.enter_context(tc.tile_pool(name="pvp", bufs=2))
        rbpool = ctx.enter_context(tc.tile_pool(name="rbp", bufs=2))
        opool = ctx.enter_context(tc.tile_pool(name="op", bufs=2))
        gate_pool = ctx.enter_context(tc.tile_pool(name="gatep", bufs=1))

        def make_tiles(b):
            Q = qpool.tile([P, N], BF16, tag="Q", name=f"Q{b}")
            Kp = kpool.tile([P, NPAD], BF16, tag=f"Kp{b % 2}", bufs=1,
                            name=f"Kp{b}")
            Vp = vpool.tile([P, NPAD], BF16, tag=f"Vp{b % 2}", bufs=1,
                            name=f"Vp{b}")
            if b < 2:
                # zero borders only (interiors fully overwritten each use):
                # top/bottom pad rows + left/right pad cols of the 64 rows.
                # b0 on Pool (its only early work), b1 on DVE (idle early).
                eng = nc.gpsimd if b == 0 else nc.vector
                for t in (Kp, Vp):
                    eng.memset(_sub(t[:], 0, [[1, PAD * WP]]), 0.0)
                    eng.memset(
                        _sub(t[:], (PAD + H) * WP, [[1, PAD * WP]]), 0.0)
                    eng.memset(
                        _sub(t[:], PAD * WP, [[WP, H], [1, PAD]]), 0.0)
                    eng.memset(
                        _sub(t[:], PAD * WP + PAD + W, [[WP, H], [1, PAD]]),
                        0.0)
            return Q, Kp, Vp

        tiles = {0: make_tiles(0), 1: make_tiles(1)}

        # ---- PE warm-up: the clock ramps only during CONTINUOUS execution
        # (0.65 -> 1.2 -> 2.4 GHz after 3us). Burn zero matmuls from t~0.3
        # so the first real GEMM runs at full speed with no busy-gap. ----
        warm = consts.tile([P, P], BF16)
        nc.vector.memset(warm[:], 0.0)
        wps = gps.tile([P, P], F32, tag="g", name="warm_ps")
        NWARM = 38
        for i in range(NWARM):
            nc.tensor.matmul(wps[:], warm[:], warm[:],
                             start=(i == 0), stop=(i == NWARM - 1))

        # ---- x streamed per quarter on ct-pinned queues; w split per
        # (branch, kind) so the first GEMM chunk starts ~3.5us in ----
        qn = N // 4
        x_sb = []
        for ct in range(4):
            xt = xpool.tile([P, N], BF16, tag=f"x{ct}", name=f"x{ct}")
            x_sb.append(xt)
        w_sb = consts.tile([P, 4, 1536], BF16)

        def _w_dma(eng, b, kind, ct):
            obase = {"q": b * P, "k": 512 + b * P, "v": 1024 + b * P}[kind]
            eng.dma_start(w_sb[:, ct, obase:obase + P],
                          wq_d[ct * P:(ct + 1) * P, obase:obase + P])

        def _x_dma(eng, ct, qtr):
            eng.dma_start(x_sb[ct][:, qtr * qn:(qtr + 1) * qn],
                          x_d[ct * P:(ct + 1) * P, qtr * qn:(qtr + 1) * qn])

        # scalar (Act) queue: ONLY 4 first-chunk DMAs (each issue costs the
        # SEQ 0.67us; anything more would starve the early k/q evacs)
        for ct in range(2):
            _x_dma(nc.scalar, ct, 0)
            _w_dma(nc.scalar, 0, "k", ct)
        # sync (SP) queue: everything else, in need order; SP.SEQ is idle
        for ct in range(2, 4):
            _x_dma(nc.sync, ct, 0)
            _w_dma(nc.sync, 0, "k", ct)
        for ct in range(4):
            _w_dma(nc.sync, 0, "q", ct)
        for ct in range(4):
            _w_dma(nc.sync, 0, "v", ct)
        bd_sb = consts.tile([P, P], BF16)
        nc.sync.dma_start(bd_sb[:], bd_d[:])
        id_sb = consts.tile([P, P], BF16)
        nc.sync.dma_start(id_sb[:], id_d[:])
        sh_sb = consts.tile([18, P], BF16)
        nc.sync.dma_start(sh_sb[:], sh_d[:])
        bg_sb = gate_pool.tile([4, 1], F32)
        nc.sync.dma_start(bg_sb[:], bg_d[:])
        for ct in range(4):
            _x_dma(nc.sync, ct, 1)
        wg_sb = consts.tile([P, 4, 4], BF16)
        for ct in range(4):
            nc.sync.dma_start(wg_sb[:, ct, :], wg_d[ct * P:(ct + 1) * P, :])
        # gpsimd (SWDGE) queue: x qtr2+3 fused (one DMA per tile — SWDGE
        # gen on the Pool engine is ~1us per DMA regardless of size); Pool
        # only has border memsets before these
        for ct in range(4):
            nc.gpsimd.dma_start(x_sb[ct][:, 2 * qn:], x_d[ct * P:(ct + 1) * P, 2 * qn:])
        for b in range(1, 4):
            for kind in ("k", "q", "v"):
                for ct in range(4):
                    _w_dma(nc.sync, b, kind, ct)

        # ---- gate pooling: wg^T x chunk matmuls (213ns each) spread
        # through s0's hook slots; ScalarE accum per chunk. The shg trick
        # keeps everything in the sps ring gate-independent, so no ring
        # deadlock is possible.
        gl_parts = gate_pool.tile([4, 8], F32)
        trash4 = gate_pool.tile([4, NQ], F32)

        def emit_gate_chunk(ch):
            ps4 = sps.tile([4, NQ], F32, tag="s", name=f"gate_ps{ch}")
            for ct in range(4):
                nc.tensor.matmul(ps4[:], wg_sb[:, ct, :],
                                 x_sb[ct][:, ch * 512:(ch + 1) * 512],
                                 start=(ct == 0), stop=(ct == 3))
            nc.scalar.activation(trash4[:], ps4[:], AF.Copy,
                                 accum_out=gl_parts[:, ch:ch + 1])

        shg = []            # per-branch gate-scaled selhead stationaries
        for b in range(4):
            shg.append(gate_pool.tile([18, P], BF16, tag=f"shg{b}",
                                      name=f"shg{b}"))

        def emit_gate_chain():
            """Softmax gate -> per-branch (1/gate_b)-scaled selhead."""
            glog4 = gate_pool.tile([4, 1], F32)
            nc.vector.reduce_sum(glog4[:], gl_parts[:], axis=mybir.AxisListType.X)
            logit4 = gate_pool.tile([4, 1], F32)
            nc.vector.scalar_tensor_tensor(logit4[:], glog4[:], 1.0 / N, bg_sb[:],
                                           op0=ALU.mult, op1=ALU.add)
            gexp4 = gate_pool.tile([4, 1], F32)
            nc.scalar.activation(gexp4[:], logit4[:], AF.Exp)
            gexpT = gate_pool.tile([1, 4], F32)
            nc.sync.dma_start(gexpT[:], gexp4[:])
            gsum = gate_pool.tile([1, 1], F32)
            nc.vector.reduce_sum(gsum[:], gexpT[:], axis=mybir.AxisListType.X)
            gexpT_rec = gate_pool.tile([1, 4], F32)
            nc.vector.reciprocal_approx_fast(gexpT_rec[:], gexpT[:])
            ginvT = gate_pool.tile([1, 4], F32)
            nc.vector.tensor_scalar_mul(ginvT[:], gexpT_rec[:], gsum[:])
            gtmp = nc.dram_tensor("gtmp", [1, 4], F32).ap()
            nc.sync.dma_start(gtmp[:], ginvT[:])
            for b in range(4):
                gb = gate_pool.tile([18, 1], F32, tag=f"ginv{b}")
                src = gtmp[0:1, b:b + 1]
                bc = bass.AP(tensor=src.tensor, offset=src.offset,
                             ap=[[0, 18], [1, 1]])
                nc.sync.dma_start(gb[:], bc)
                nc.vector.tensor_scalar_mul(shg[b][:], sh_sb[:], gb[:])

        # exp grouping: single tap first so the PSUM pair-slot ring never
        # makes a late matmul wait on a pending exp
        PAIRS = [(8,), (0, 1), (2, 3), (4, 5), (6, 7)]

        def emit_gemm_chunk(b, kind, ch):
            if b not in tiles:
                tiles[b] = make_tiles(b)
            Q, Kp, Vp = tiles[b]
            obase = {"q": b * P, "k": 512 + b * P, "v": 1024 + b * P}[kind]
            ps = gps.tile([P, 512], F32, tag="g", name=f"g_{b}_{kind}_{ch}")
            for ct in range(4):
                nc.tensor.matmul(
                    ps[:], w_sb[:, ct, obase:obase + P],
                    x_sb[ct][:, ch * 512:(ch + 1) * 512],
                    start=(ct == 0), stop=(ct == 3))
            if kind == "q":
                nc.scalar.activation(Q[:, ch * 512:(ch + 1) * 512],
                                     ps[:], AF.Copy)
            else:
                dst_t = Kp if kind == "k" else Vp
                dst = _sub(dst_t[:], (PAD + ch * QROWS) * WP + PAD,
                           [[WP, QROWS], [1, W]])
                ps_v = ps[:].rearrange("p (r c) -> p r c", c=W)
                if kind == "k":
                    nc.scalar.activation(dst, ps_v, AF.Copy)
                else:
                    # V evac on DVE to offload ScalarE
                    nc.vector.tensor_scalar_mul(dst, ps_v, 1.0)

        chunk_q = []
        for b in range(4):
            chunk_q.extend(_need_order(b))

        def pull(n):
            for _ in range(n):
                if chunk_q:
                    b, kind, ch = chunk_q.pop(0)
                    emit_gemm_chunk(b, kind, ch)

        def emit_prod(b, dil, s):
            """Stage A: fused QK products for step s (one step ahead)."""
            Q, Kp, _ = tiles[b]
            r0 = s * QROWS
            prod = prodp.tile([P, 9, NQ], BF16, tag="prod",
                              name=f"prod_{b}_{s}")
            for di in range(3):
                # b0's first steps: all-DVE (Pool still busy with memsets
                # + SWDGE descriptor gen for the x loads)
                eng = nc.gpsimd if di in QK_POOL_DI and not (b == 0 and s < 2) \
                    else nc.vector
                koff = (PAD + r0 + (di - 1) * dil) * WP + (PAD - dil)
                in1 = _sub(Kp[:], koff, [[dil, 3], [WP, QROWS], [1, W]])
                in0 = _sub(Q[:], r0 * W, [[0, 3], [W, QROWS], [1, W]])
                pout = _sub(prod[:], 3 * di * NQ, [[NQ, 3], [W, QROWS], [1, W]])
                eng.tensor_mul(pout, in0, in1)
            return prod

        pending_dr = []     # states awaiting D+Rb (pi1 slot)
        pending_av = []     # states awaiting AV+normalize+store (pi3 slot)
        n_stores = [0]

        def emit_pv_di(b, dil, s, Edall, pv, di):
            """Stage C1: E*v_shift product for one tap-row group, emitted as
            soon as its exp pairs are done so DVE leads PE."""
            Vp = tiles[b][2]
            r0 = s * QROWS
            eng = nc.gpsimd if (di in AV_POOL_DI or (b == 3 and di == 1)) \
                else nc.vector
            voff = (PAD + r0 + (di - 1) * dil) * WP + (PAD - dil)
            vin1 = _sub(Vp[:], voff, [[dil, 3], [WP, QROWS], [1, W]])
            ein0 = _sub(Edall[:], 3 * di * NQ, [[NQ, 3], [W, QROWS], [1, W]])
            pvout = _sub(pv[:], 3 * di * NQ, [[NQ, 3], [W, QROWS], [1, W]])
            eng.tensor_mul(pvout, ein0, vin1)

        def emit_dr(st):
            """Stage C2a (pi1): denominator matmul + reciprocal. One slot
            ahead of the AV/osb so neither engine blocks on the other."""
            b, dil, s, pv, Ec = st
            D = sps.tile([P, NQ], F32, tag="s", name=f"D_{b}_{s}")
            nc.tensor.matmul(D[:], shg[b][:], Ec[:], start=True, stop=True)
            Rb = rbpool.tile([P, NQ], F32, tag="rb", name=f"Rb_{b}_{s}")
            nc.vector.reciprocal_approx_fast(Rb[:], D[:])
            return (b, dil, s, pv, Rb)

        def emit_av(st):
            """Stage C2b (post-pairs): AV accumulate, normalize, store.
            Taps computed on the slower Pool engine accumulate LAST."""
            b, dil, s, pv, Rb = st
            order = [t for di in range(3)
                     if not (di in AV_POOL_DI or (b == 3 and di == 1))
                     for t in range(3 * di, 3 * di + 3)]
            order += [t for t in range(9) if t not in order]
            AV = sps.tile([P, NQ], F32, tag="s", name=f"AV_{b}_{s}")
            for i, t in enumerate(order):
                nc.tensor.matmul(AV[:], id_sb[:],
                                 _sub(pv[:], t * NQ, [[1, NQ]]),
                                 start=(i == 0), stop=(i == 8))
            osb = opool.tile([P, NQ], F32, tag="osb", name=f"osb_{b}_{s}")
            nc.vector.tensor_mul(osb[:], AV[:], Rb[:])
            (nc.sync if n_stores[0] % 2 == 0 else nc.scalar).dma_start(
                out_d[b * P:(b + 1) * P, s * NQ:(s + 1) * NQ], osb[:])
            n_stores[0] += 1

        def flush_dr():
            if pending_dr:
                pending_av.append(emit_dr(pending_dr.pop(0)))

        def flush_av():
            if pending_av:
                emit_av(pending_av.pop(0))

        prev_prod = {}      # per-branch prefetched stage-A product

        def emit_step(b, dil, s, pulls, hook_pi0=None, hook_pi3=None,
                      hook_end=None, no_dr=False):
            """One attention step: logits pairs + exp, pipelined stage C of
            earlier steps (D+Rb at pi1, AV+store at pi3), stage-A prefetch,
            GEMM-chunk filler, Ec gather + pv at step end."""
            prod = prev_prod[b]
            Edall = epool.tile([P, 9, NQ], BF16, tag="Edall",
                               name=f"Edall_{b}_{s}")
            pv = pvp.tile([P, 9, NQ], BF16, tag="pv", name=f"pv_{b}_{s}")
            n_front = pulls // 2
            for pi, pair in enumerate(PAIRS):
                npair = len(pair)
                Ld = ldps.tile([P, npair, NQ], F32, tag="ld",
                               name=f"Ld_{b}_{s}_{pi}")
                for j, t in enumerate(pair):
                    nc.tensor.matmul(Ld[:, j, :], bd_sb[:],
                                     _sub(prod[:], t * NQ, [[1, NQ]]),
                                     start=True, stop=True)
                nc.scalar.activation(Edall[:, pair[0]:pair[0] + npair, :],
                                     Ld[:], AF.Exp, scale=0.125)
                if pi == 0 and hook_pi0 is not None:
                    hook_pi0()
                elif pi == 1:
                    if not no_dr:
                        flush_dr()
                    # stage A: products for next step (keeps DVE ahead)
                    if s + 1 < NSTEP:
                        prev_prod[b] = emit_prod(b, dil, s + 1)
                elif pi == 2:
                    # taps 0-2 exp'd (pairs 1+2) -> their E*v can start
                    emit_pv_di(b, dil, s, Edall, pv, 0)
                elif pi == 3:
                    emit_pv_di(b, dil, s, Edall, pv, 1)
                    if hook_pi3 is not None:
                        hook_pi3()

            # AV/osb/chunks after ALL pairs: the next step's pair-1 logit
            # matmul needs this step's pair-4 exp (PSUM ring), so pair-4
            # must not sit behind this ~3us block
            flush_av()
            pull(n_front)

            # ---- compact denominator gather ----
            Ec = ecpool.tile([18, NQ], BF16, tag="ec", name=f"Ec_{b}_{s}")
            esrc = bass.AP(tensor=Edall.tensor, offset=Edall[:].offset,
                           ap=[[64 * EPITCH, 2], [NQ, 9], [1, NQ]])
            nc.sync.dma_start(Ec[:], esrc)
            pull(pulls - n_front)
            emit_pv_di(b, dil, s, Edall, pv, 2)
            pending_dr.append((b, dil, s, pv, Ec))
            if hook_end is not None:
                hook_end()

        # ---- schedule: head chunks (+gate qtr0 accums), b0/b1 solo with
        # gate pooling in idle slots, then b2+b3 merged ----
        pull(10)
        emit_gate_chunk(0)
        emit_gate_chunk(1)
        prev_prod[0] = emit_prod(0, DILS[0], 0)
        emit_step(0, DILS[0], 0, 3,
                  hook_pi0=lambda: (emit_gate_chunk(2), emit_gate_chunk(3)),
                  hook_pi3=lambda: (emit_gate_chunk(4), emit_gate_chunk(5)),
                  hook_end=lambda: (emit_gate_chunk(6), emit_gate_chunk(7)))
        emit_step(0, DILS[0], 1, 3, no_dr=True,
                  hook_pi0=emit_gate_chain)
        for s in range(2, NSTEP):
            emit_step(0, DILS[0], s, 3)
        prev_prod[1] = emit_prod(1, DILS[1], 0)
        for s in range(NSTEP):
            emit_step(1, DILS[1], s, 4)
        prev_prod[2] = emit_prod(2, DILS[2], 0)
        MERGED_PULLS = [5, 4, 4, 4, 4, 4, 3, 3]
        for ms in range(NSTEP + 2):
            if ms < NSTEP:
                emit_step(2, DILS[2], ms, MERGED_PULLS[ms])
            if ms == 2:
                prev_prod[3] = emit_prod(3, DILS[3], 0)
            if ms >= 2:
                emit_step(3, DILS[3], ms - 2, 0)
        while pending_av or pending_dr:
            flush_av()
            flush_dr()

    nc.compile()
    return nc


def _consts():
    bf = ml_dtypes.bfloat16
    blkdiag = np.zeros((128, 128), np.float32)
    blkdiag[:64, :64] = 1.0
    blkdiag[64:, 64:] = 1.0
    selhead = np.zeros((18, 128), np.float32)
    selhead[:9, :64] = 1.0
    selhead[9:, 64:] = 1.0
    ident = np.eye(128, dtype=np.float32)
    return (blkdiag.astype(bf), selhead.astype(bf), ident.astype(bf))


def kernel(x, w_qkv, w_gate, b_gate):
    x = np.asarray(x, dtype=np.float32)
    w_qkv = np.asarray(w_qkv, dtype=np.float32)
    w_gate = np.asarray(w_gate, dtype=np.float32)
    b_gate = np.asarray(b_gate, dtype=np.float32)
    if "nc" not in _CACHE:
        _CACHE["nc"] = _build()
    nc = _CACHE["nc"]
    blkdiag, selhead, ident = _consts()
    wqkvT = np.ascontiguousarray(w_qkv.T).astype(ml_dtypes.bfloat16)
    wgT = np.ascontiguousarray(w_gate.T).astype(ml_dtypes.bfloat16)
    bg = b_gate.reshape(4, 1).astype(np.float32)
    in_maps = []
    for b in range(8):
        in_maps.append({
            "x": np.ascontiguousarray(x[b].reshape(512, N)).astype(ml_dtypes.bfloat16),
            "wqkvT": wqkvT, "wgT": wgT, "bg": bg,
            "blkdiag": blkdiag, "selhead": selhead, "ident": ident,
        })
    res = run_bass_kernel_spmd(nc, in_maps, core_ids=list(range(8)),
                               trace=bool(int(os.environ.get("KTRACE", "0"))))
    _CACHE["last"] = res
    out = np.stack([np.asarray(res.results[b]["out"], dtype=np.float32)
                    .reshape(512, H, W) for b in range(8)])
    return out
